# revision 1
# baseline (speedup 1.0000x reference)
"""DeltaRule memory scan kernel for Trainium2, 8 NeuronCores, data-parallel over batch.

Reference semantics (per batch element, H=512, L=2048):
    M_0 = 0  [H,H]
    for t in 0..L-2:   k = hidden[t]
        d = k.k + eps; delta = k - (M k)/d; M += outer(delta, k)
    out = (M @ hidden[L-1]) @ W.T + b

Implementation: chunked delta rule (UT transform), chunk C=128.
Per chunk with keys K [C,H], r = 1/(rowsum(K^2)+eps):
    A  = strict_tril(diag(r) K K^T)            [C,C]
    T  = (I+A)^{-1} ~= (I-A)(I+A^2)(I+A^4)(I+A^8)(I+A^16)   (A nilpotent, ||A||<1)
    U  = K - diag(r) (K M^T)                   [C,H]
    Dl = T U                                    [C,H]
    M^T += K^T Dl
C x C chain runs in bf16 on the PE; state matmuls run as float32r (full fp32
inputs, 1 cycle/row). 4 batch elements per core, chunk loops interleaved across
batch for latency hiding.
"""
import sys
import numpy as np
from contextlib import ExitStack

sys.path.insert(0, "/opt/trn_rl_repo")

import concourse.bass as bass
import concourse.mybir as mybir
from concourse import tile
from concourse.bass_utils import run_bass_kernel_spmd
from concourse.masks import make_identity, make_lower_triangular

B, L, H = 32, 2048, 512
NCORES = 8
BPC = B // NCORES          # 4 batch elements per core
C = 128                    # chunk size
T = L - 1                  # 2047 scan steps
NCHUNK = (T + C - 1) // C  # 16 (last chunk has 127 valid rows)
NLEV = 4                   # Neumann levels -> exact through A^31, error ~||A||^32
EPS = 1e-6
HB = H // 128              # 4 h-blocks

f32 = mybir.dt.float32
f32r = mybir.dt.float32r
bf16 = mybir.dt.bfloat16

_cached = {}


def _build_program():
    nc = bass.Bass(target_bir_lowering=False, debug=False)

    hidden_d = nc.declare_dram_parameter("hidden", [BPC, L, H], f32, isOutput=False)
    w_d = nc.declare_dram_parameter("W", [H, H], f32, isOutput=False)
    b_d = nc.declare_dram_parameter("bvec", [H], f32, isOutput=False)
    out_d = nc.declare_dram_parameter("out", [BPC, H], f32, isOutput=True)

    with tile.TileContext(nc) as tc, ExitStack() as ctx:
        consts = ctx.enter_context(tc.tile_pool(name="consts", bufs=1))
        wbuild = ctx.enter_context(tc.tile_pool(name="wbuild", bufs=2))
        mtpool = ctx.enter_context(tc.tile_pool(name="mt", bufs=1))
        kpool = ctx.enter_context(tc.tile_pool(name="k", bufs=8))
        ktpool = ctx.enter_context(tc.tile_pool(name="kt", bufs=8))
        chain = ctx.enter_context(tc.tile_pool(name="chain", bufs=8))
        upool = ctx.enter_context(tc.tile_pool(name="u", bufs=8))
        small = ctx.enter_context(tc.tile_pool(name="small", bufs=8))
        pslo = ctx.enter_context(tc.tile_pool(name="pslo", bufs=2, space="PSUM"))
        pshi = ctx.enter_context(tc.tile_pool(name="pshi", bufs=6, space="PSUM"))

        # ---- constants ----
        ident_f = consts.tile([128, 128], f32, tag="identf")
        make_identity(nc, ident_f[:])
        ident_b = consts.tile([128, 128], bf16, tag="identb")
        make_identity(nc, ident_b[:])
        # packed identity: I in each of the 4 b-slices
        identp = consts.tile([128, H], bf16, tag="identp")
        for bi in range(BPC):
            nc.vector.tensor_copy(identp[:, bi * 128:(bi + 1) * 128], ident_b[:])
        smask = consts.tile([128, 128], f32, tag="smask")
        make_lower_triangular(nc, smask[:], val=1.0, diag=False)

        # W^T: WT[ib][i', o] = W[o, ib*128+i']  (f32, used once in the finale)
        wt = [consts.tile([128, H], f32, tag=f"wt{ib}", name=f"wt{ib}") for ib in range(HB)]
        for op in range(HB):
            wsb = wbuild.tile([128, H], f32, tag="wsb")
            nc.sync.dma_start(wsb[:], w_d[op * 128:(op + 1) * 128, :])
            for ib in range(HB):
                tps = pslo.tile([128, 128], f32, tag="sm")
                nc.tensor.transpose(tps[:], wsb[:, ib * 128:(ib + 1) * 128], ident_f[:])
                nc.scalar.copy(wt[ib][:, op * 128:(op + 1) * 128], tps[:])

        bias_row = consts.tile([1, H], f32, tag="biasrow")
        nc.sync.dma_start(bias_row[:], b_d[None, :])

        # q[b] as [128, HB] column tile (q_t[p, jb] = q[jb*128+p])
        qs = []
        for bi in range(BPC):
            v4 = wbuild.tile([HB, 128], f32, tag="v4")
            nc.sync.dma_start(v4[:], hidden_d[bi, L - 1, :].rearrange("(f p) -> f p", p=128))
            tps = pslo.tile([128, HB], f32, tag="sm")
            nc.tensor.transpose(tps[:], v4[:], ident_f[:HB, :HB])
            q_t = consts.tile([128, HB], f32, tag=f"q{bi}", name=f"q{bi}")
            nc.scalar.copy(q_t[:], tps[:])
            qs.append(q_t)

        # ---- state: M^T per (b, jb): f32 accumulator + bf16 matmul copy ----
        mts = [[mtpool.tile([128, H], f32, tag=f"mt{bi}_{jb}", name=f"mt{bi}_{jb}")
                for jb in range(HB)] for bi in range(BPC)]
        mtbs = [[mtpool.tile([128, H], bf16, tag=f"mtb{bi}_{jb}", name=f"mtb{bi}_{jb}")
                 for jb in range(HB)] for bi in range(BPC)]

        # ---- software-pipelined main loop ----
        # prep/aform/chain for group c+1 are interleaved into the state section
        # of group c so the PE queue never stalls behind ACT/DVE turnarounds.
        G = {}  # per-group live tiles

        def prep(c):
            t0 = c * C
            nrows = min(C, T - t0)
            st = {"k": [], "kb": [], "ktb": [], "r": [], "nr": []}
            for bi in range(BPC):
                k_t = kpool.tile([128, H], f32, tag="K", name=f"k{c}_{bi}")
                if nrows < C:
                    nc.vector.memset(k_t[:], 0.0)
                    nc.sync.dma_start(k_t[:nrows, :], hidden_d[bi, t0:t0 + nrows, :])
                else:
                    nc.sync.dma_start(k_t[:], hidden_d[bi, t0:t0 + C, :])
                st["k"].append(k_t)
                scr = small.tile([128, H], bf16, tag="scr")
                d_t = small.tile([128, 1], f32, tag="d")
                nc.scalar.activation(scr[:], k_t[:], mybir.ActivationFunctionType.Square,
                                     accum_out=d_t[:])
                r_t = small.tile([128, 1], f32, tag="r")
                nc.vector.tensor_scalar_add(d_t[:], d_t[:], EPS)
                nc.vector.reciprocal(r_t[:], d_t[:])
                nr_t = small.tile([128, 1], f32, tag="nr")
                nc.vector.tensor_scalar_mul(nr_t[:], r_t[:], -1.0)
                st["r"].append(r_t); st["nr"].append(nr_t)
                kb = kpool.tile([128, H], bf16, tag="Kb", name=f"kb{c}_{bi}")
                nc.scalar.copy(kb[:], k_t[:])
                st["kb"].append(kb)
                ktps = pshi.tile([128, H], bf16, tag="big")
                for hb in range(HB):
                    nc.tensor.transpose(ktps[:, hb * 128:(hb + 1) * 128],
                                        kb[:, hb * 128:(hb + 1) * 128], ident_b[:])
                ktb = ktpool.tile([128, H], bf16, tag="ktb", name=f"ktb{c}_{bi}")
                nc.scalar.copy(ktb[:], ktps[:])
                st["ktb"].append(ktb)
            G[c] = st

        def aform(c):
            st = G[c]
            a_ps = pshi.tile([128, H], f32, tag="big")
            for bi in range(BPC):
                sl = slice(bi * 128, (bi + 1) * 128)
                for hb in range(HB):
                    nc.tensor.matmul(a_ps[:, sl], st["ktb"][bi][:, hb * 128:(hb + 1) * 128],
                                     st["ktb"][bi][:, hb * 128:(hb + 1) * 128],
                                     start=(hb == 0), stop=(hb == HB - 1))
            a_all = chain.tile([128, H], bf16, tag="ak")
            for bi in range(BPC):
                sl = slice(bi * 128, (bi + 1) * 128)
                nc.vector.scalar_tensor_tensor(a_all[:, sl], a_ps[:, sl], st["r"][bi][:],
                                               smask[:], mybir.AluOpType.mult,
                                               mybir.AluOpType.mult)
            at_ps = pshi.tile([128, H], bf16, tag="big")
            for bi in range(BPC):
                sl = slice(bi * 128, (bi + 1) * 128)
                nc.tensor.transpose(at_ps[:, sl], a_all[:, sl], ident_b[:])
            at_all = chain.tile([128, H], bf16, tag="atk")
            nc.scalar.copy(at_all[:], at_ps[:])
            g0 = chain.tile([128, H], bf16, tag="g")
            nc.vector.tensor_sub(g0[:], identp[:], at_all[:])
            st["ak"], st["atk"], st["g"] = a_all, at_all, g0

        def chain_level(c, lev):
            st = G[c]
            ak, atk = st["ak"], st["atk"]
            sq1 = pshi.tile([128, H], f32, tag="big")
            for bi in range(BPC):
                sl = slice(bi * 128, (bi + 1) * 128)
                nc.tensor.matmul(sq1[:, sl], atk[:, sl], ak[:, sl], start=True, stop=True)
            ak2 = chain.tile([128, H], bf16, tag="ak")
            nc.scalar.copy(ak2[:], sq1[:])
            if lev < NLEV:
                sq2 = pshi.tile([128, H], f32, tag="big")
                for bi in range(BPC):
                    sl = slice(bi * 128, (bi + 1) * 128)
                    nc.tensor.matmul(sq2[:, sl], ak[:, sl], atk[:, sl], start=True, stop=True)
                atk2 = chain.tile([128, H], bf16, tag="atk")
                nc.scalar.copy(atk2[:], sq2[:])
            else:
                atk2 = None
            gps = pshi.tile([128, H], f32, tag="big")
            for bi in range(BPC):
                sl = slice(bi * 128, (bi + 1) * 128)
                nc.tensor.matmul(gps[:, sl], ak2[:, sl], st["g"][:, sl], start=True, stop=True)
            g_nxt = chain.tile([128, H], bf16, tag="g")
            nc.vector.tensor_add(g_nxt[:], gps[:], st["g"][:])
            st["ak"], st["atk"], st["g"] = ak2, atk2, g_nxt

        def state_u(c):
            st = G[c]
            st["u"] = []
            for bi in range(BPC):
                if c == 0:
                    st["u"].append(st["kb"][bi])
                    continue
                ups = pshi.tile([128, H], f32, tag="big")
                for hb in range(HB):
                    nc.tensor.matmul(ups[:], st["ktb"][bi][:, hb * 128:(hb + 1) * 128],
                                     mtbs[bi][hb][:],
                                     start=(hb == 0), stop=(hb == HB - 1))
                u_sb = upool.tile([128, H], bf16, tag="u")
                nc.vector.scalar_tensor_tensor(u_sb[:], ups[:], st["nr"][bi][:],
                                               st["k"][bi][:], mybir.AluOpType.mult,
                                               mybir.AluOpType.add)
                st["u"].append(u_sb)

        def state_delta(c):
            st = G[c]
            st["dl"] = []
            for bi in range(BPC):
                sl = slice(bi * 128, (bi + 1) * 128)
                dps = pshi.tile([128, H], f32, tag="big")
                nc.tensor.matmul(dps[:], st["g"][:, sl], st["u"][bi][:], start=True, stop=True)
                dl_sb = upool.tile([128, H], bf16, tag="dl")
                nc.scalar.copy(dl_sb[:], dps[:])
                st["dl"].append(dl_sb)

        def state_mupd(c, bis):
            st = G[c]
            for bi in bis:
                for jb in range(HB):
                    mps = pshi.tile([128, H], f32, tag="big")
                    nc.tensor.matmul(mps[:], st["kb"][bi][:, jb * 128:(jb + 1) * 128],
                                     st["dl"][bi][:], start=True, stop=True)
                    if c == 0:
                        nc.vector.tensor_copy(mts[bi][jb][:], mps[:])
                    else:
                        nc.vector.tensor_add(mts[bi][jb][:], mps[:], mts[bi][jb][:])
                    if jb % 2 == 0:
                        nc.scalar.copy(mtbs[bi][jb][:], mts[bi][jb][:])
                    else:
                        nc.vector.tensor_copy(mtbs[bi][jb][:], mts[bi][jb][:])

        prep(0)
        aform(0)
        for lev in range(1, NLEV + 1):
            chain_level(0, lev)
        for c in range(NCHUNK):
            nxt = c + 1 if c + 1 < NCHUNK else None
            if nxt is not None:
                prep(nxt)
                aform(nxt)
            state_u(c)
            if nxt is not None:
                chain_level(nxt, 1)
            state_delta(c)
            if nxt is not None:
                chain_level(nxt, 2)
            state_mupd(c, [0, 1])
            if nxt is not None:
                chain_level(nxt, 3)
            state_mupd(c, [2, 3])
            if nxt is not None:
                chain_level(nxt, 4)
            prev = c - 1
            if prev in G:
                del G[prev]

        # ---- finale: ctx = M q (row form); out = ctx W^T + b ----
        for bi in range(BPC):
            # ctx_row[0, i] = sum_j q[j] MT[j, i]
            cps = pshi.tile([1, H], f32, tag="big")
            for jb in range(HB):
                nc.tensor.matmul(cps[:], qs[bi][:, jb:jb + 1],
                                 mts[bi][jb][:],
                                 start=(jb == 0), stop=(jb == HB - 1))
            ctx_row = small.tile([1, H], f32, tag="ctxrow")
            nc.scalar.copy(ctx_row[:], cps[:])
            # ctxT [128, HB]: 4 tiny transposes
            ctxT = small.tile([128, HB], f32, tag="ctxT")
            for ib in range(HB):
                tp2 = pslo.tile([128, 1], f32, tag="sm")
                nc.tensor.transpose(tp2[:], ctx_row[:, ib * 128:(ib + 1) * 128], ident_f[:1, :1])
                nc.scalar.copy(ctxT[:, ib:ib + 1], tp2[:])
            # out_row[0, o] = sum_i ctxT[i] W[o, i] + b[o]
            ops_ = pshi.tile([1, H], f32, tag="big")
            for ib in range(HB):
                nc.tensor.matmul(ops_[:], ctxT[:, ib:ib + 1], wt[ib][:],
                                 start=(ib == 0), stop=(ib == HB - 1))
            out_row = small.tile([1, H], f32, tag="outrow")
            nc.vector.tensor_add(out_row[:], ops_[:], bias_row[:])
            nc.sync.dma_start(out_d[bi, :][None, :], out_row[:])

    _legalize_waits(nc)
    return nc


def _legalize_waits(nc, max_waits=1):
    """This toolchain's walrus encodes at most one semaphore wait per
    instruction. Hoist extra waits onto standalone EventSemaphore
    instructions on the same engine queue, immediately before the owner."""
    import json as _json
    m = _json.loads(bytes(nc.to_json_bytes()))
    n_fix = 0
    for fn in m["functions"]:
        for blk in fn["blocks"]:
            out = []
            for ins in blk.get("instructions", []):
                si = ins.get("sync_info") or {}
                waits = si.get("on_wait") or []
                if len(waits) > max_waits and ins.get("opcode") != "EventSemaphore":
                    extra, keep = waits[:-max_waits], waits[-max_waits:]
                    for i, w in enumerate(extra):
                        out.append({
                            "name": f"{ins['name']}-w{i}",
                            "engine": ins["engine"],
                            "opcode": "EventSemaphore",
                            "ins": [], "outs": [],
                            "sync_info": {"on_wait": [w], "on_update": []},
                        })
                    si["on_wait"] = keep
                    ins["sync_info"] = si
                    n_fix += 1
                out.append(ins)
            blk["instructions"] = out
    nc.m = mybir.module_from_json_bytes(_json.dumps(m).encode())
    return n_fix


def kernel(hidden: np.ndarray, W: np.ndarray, b: np.ndarray) -> np.ndarray:
    if "nc" not in _cached:
        _cached["nc"] = _build_program()
    nc = _cached["nc"]

    hidden = np.ascontiguousarray(hidden, dtype=np.float32)
    W = np.ascontiguousarray(W, dtype=np.float32)
    b = np.ascontiguousarray(b, dtype=np.float32)

    in_maps = []
    for ci in range(NCORES):
        in_maps.append({
            "hidden": hidden[ci * BPC:(ci + 1) * BPC],
            "W": W,
            "bvec": b,
        })
    res = run_bass_kernel_spmd(nc, in_maps, core_ids=list(range(NCORES)))
    _cached["last_results"] = res
    out = np.concatenate([res.results[ci]["out"] for ci in range(NCORES)], axis=0)
    return out.astype(np.float32)


if __name__ == "__main__":
    rng = np.random.default_rng(0)
    h = rng.standard_normal((B, L, H), dtype=np.float32)
    w = rng.standard_normal((H, H), dtype=np.float32) * (1.0 / np.sqrt(H))
    bb = np.zeros((H,), np.float32)
    o = kernel(h, w, bb)
    print(o.shape, o.dtype)



# revision 2
# speedup vs baseline: 1.1178x; 1.1178x over previous
"""DeltaRule memory via backward adjoint solve. Trainium2, 8 cores, DP over batch.

Key identity: the output only needs ctx = M_final @ q. With M_0 = 0,
    ctx = K^T y,   (I + A^T) y = w,   w = K q,   A_ts = r_t (k_t.k_s), s < t
so the whole scan collapses to one triangular solve with a single RHS,
processed backward in C=128 chunks with vector state m = sum r_s y_s k_s:
    rhs_c = K_c (q - m)
    y_c   = T_c^T rhs_c,   T_c = (I + A_cc)^{-1} = (I-A)(I+A^2)(I+A^4)(I+A^8)
    m    += K_c^T (r_c * y_c);   ctx += K_c^T y_c
Per-chunk prep (K load/cast/transpose, S = K K^T, A masks, squared powers
B2/B4/B8) is order-independent and pipelined ahead; only tiny [128,4]
vector ops sit on the serial path. All matmuls bf16 (rel err ~9e-3).
"""
import sys
import numpy as np
from contextlib import ExitStack

sys.path.insert(0, "/opt/trn_rl_repo")

import concourse.bass as bass
import concourse.mybir as mybir
from concourse import tile
from concourse.bass_utils import run_bass_kernel_spmd
from concourse.masks import make_identity, make_lower_triangular

B, L, H = 32, 2048, 512
NCORES = 8
BPC = B // NCORES          # 4 batch elements per core
C = 128                    # chunk size
T = L - 1                  # 2047 scan steps
NCHUNK = (T + C - 1) // C  # 16 (last chunk has 127 valid rows)
NLEV = 2                   # Neumann factors through (I+A^4): exact to A^7
EPS = 1e-6
HB = H // 128              # 4 h-blocks
PK = H * BPC               # 2048: packed free dim (4 batches x 512)

f32 = mybir.dt.float32
f32r = mybir.dt.float32r
bf16 = mybir.dt.bfloat16

_cached = {}


def _build_program(legalize=True):
    nc = bass.Bass(target_bir_lowering=False, debug=False)

    hidden_d = nc.declare_dram_parameter("hidden", [BPC, L, H], f32, isOutput=False)
    w_d = nc.declare_dram_parameter("W", [H, H], f32, isOutput=False)
    b_d = nc.declare_dram_parameter("bvec", [H], f32, isOutput=False)
    out_d = nc.declare_dram_parameter("out", [BPC, H], f32, isOutput=True)

    with tile.TileContext(nc) as tc, ExitStack() as ctx:
        consts = ctx.enter_context(tc.tile_pool(name="consts", bufs=1))
        wbuild = ctx.enter_context(tc.tile_pool(name="wbuild", bufs=2))
        kf = ctx.enter_context(tc.tile_pool(name="kf", bufs=4))
        kb = ctx.enter_context(tc.tile_pool(name="kb", bufs=6))
        ktb = ctx.enter_context(tc.tile_pool(name="ktb", bufs=6))
        chl = ctx.enter_context(tc.tile_pool(name="chl", bufs=6))   # A,B2,B4,B8
        chs = ctx.enter_context(tc.tile_pool(name="chs", bufs=4))   # At,B2t
        small = ctx.enter_context(tc.tile_pool(name="small", bufs=6))
        vec = ctx.enter_context(tc.tile_pool(name="vec", bufs=10))
        psS = ctx.enter_context(tc.tile_pool(name="psS", bufs=2, space="PSUM"))
        psq = ctx.enter_context(tc.tile_pool(name="psq", bufs=2, space="PSUM"))
        pst = ctx.enter_context(tc.tile_pool(name="pst", bufs=3, space="PSUM"))
        psv = ctx.enter_context(tc.tile_pool(name="psv", bufs=1, space="PSUM"))

        # ---- constants ----
        ident_f = consts.tile([128, 128], f32, tag="identf")
        make_identity(nc, ident_f[:])
        ident_b = consts.tile([128, 128], bf16, tag="identb")
        make_identity(nc, ident_b[:])
        identp = consts.tile([128, H], bf16, tag="identp")  # I per 128-col slice
        for bi in range(BPC):
            nc.vector.tensor_copy(identp[:, bi * 128:(bi + 1) * 128], ident_b[:])
        smask = consts.tile([128, 128], f32, tag="smask")   # strict lower ones
        make_lower_triangular(nc, smask[:], val=1.0, diag=False)

        # W^T tiles in f32r: wt[ib][i', o] = W[o, ib*128+i'] (emitted mid-sweep)
        wt = [consts.tile([128, H], f32r, tag=f"wt{ib}", name=f"wt{ib}")
              for ib in range(HB)]
        bias_row = consts.tile([1, H], f32, tag="biasrow")

        def wbuild_stages():
            out = []

            def s_w(op):
                def fn():
                    wsb = wbuild.tile([128, H], f32, tag="wsb")
                    nc.scalar.dma_start(wsb[:], w_d[op * 128:(op + 1) * 128, :])
                    for ib in range(HB):
                        tps = psq.tile([128, H], f32, tag="sq")
                        nc.tensor.transpose(tps[:, :128],
                                            wsb[:, ib * 128:(ib + 1) * 128], ident_f[:])
                        nc.scalar.copy(wt[ib][:, op * 128:(op + 1) * 128], tps[:, :128])
                return fn
            for op in range(HB):
                out.append(s_w(op))

            def s_bias():
                nc.scalar.dma_start(bias_row[:], b_d[None, :])
            out.append(s_bias)
            return out

        # q per batch as [128, HB] columns, packed [128, 16] f32
        qcol = consts.tile([128, HB * BPC], f32, tag="qcol")
        for bi in range(BPC):
            v4 = wbuild.tile([HB, 128], f32, tag="v4")
            nc.sync.dma_start(v4[:], hidden_d[bi, L - 1, :].rearrange("(f p) -> f p", p=128))
            tps = psv.tile([128, 128], f32, tag="pv")
            nc.tensor.transpose(tps[:, :HB], v4[:], ident_f[:HB, :HB])
            nc.scalar.copy(qcol[:, bi * HB:(bi + 1) * HB], tps[:, :HB])

        # ---- running vector state (per batch columns packed) ----
        mcol = consts.tile([128, HB * BPC], f32, tag="mcol")
        nc.vector.memset(mcol[:], 0.0)
        ctxcol = consts.tile([128, HB * BPC], f32, tag="ctxcol")
        nc.vector.memset(ctxcol[:], 0.0)

        G = {}

        def prep_stages(c):
            """Return a list of closures emitting chunk-c prep instructions."""
            t0 = c * C
            nrows = min(C, T - t0)
            st = {}
            G[c] = st
            out = []

            def s_load():
                kfull = kf.tile([128, PK], f32, tag="kf", name=f"kf{c}")
                st["kf"] = kfull
                for bi in range(BPC):
                    eng = nc.sync if bi % 2 == 0 else nc.scalar
                    sl = slice(bi * H, (bi + 1) * H)
                    if nrows < C:
                        eng.dma_start(kfull[:nrows, sl], hidden_d[bi, t0:t0 + nrows, :])
                    else:
                        eng.dma_start(kfull[:, sl], hidden_d[bi, t0:t0 + C, :])
            out.append(s_load)

            def s_cast():
                kbb = kb.tile([128, PK], bf16, tag="kb", name=f"kb{c}")
                st["kb"] = kbb
                if nrows < C:
                    nc.vector.memset(kbb[:], 0.0)
                    nc.scalar.copy(kbb[:nrows, :], st["kf"][:nrows, :])
                else:
                    nc.scalar.copy(kbb[:], st["kf"][:])
            out.append(s_cast)

            def s_ktr(half, eng):
                def fn():
                    if half == 0:
                        ktbb = ktb.tile([128, PK], bf16, tag="ktb", name=f"ktb{c}")
                        st["ktb"] = ktbb
                    ktbb = st["ktb"]
                    ktp = pst.tile([128, PK // 2], bf16, tag="ktp")
                    for i in range(PK // 256):
                        off = half * (PK // 2) + i * 128
                        nc.tensor.transpose(ktp[:, i * 128:(i + 1) * 128],
                                            st["kb"][:, off:off + 128], ident_b[:])
                    dst = ktbb[:, half * (PK // 2):(half + 1) * (PK // 2)]
                    if eng is nc.scalar:
                        nc.scalar.copy(dst, ktp[:])
                    else:
                        nc.vector.tensor_copy(dst, ktp[:])
                return fn
            out.append(s_ktr(0, nc.scalar))
            out.append(s_ktr(1, nc.vector))

            def s_r():
                # d = rowsum(kbb^2) + EPS per batch via DVE square + 3D reduce
                sqt = small.tile([128, PK], bf16, tag="sqt")
                nc.vector.tensor_tensor(sqt[:], st["kb"][:], st["kb"][:],
                                        mybir.AluOpType.mult)
                dcol = vec.tile([128, BPC], f32, tag="dcol")
                nc.vector.tensor_reduce(
                    dcol[:], sqt[:].rearrange("p (b x) -> p b x", b=BPC),
                    mybir.AxisListType.X, mybir.AluOpType.add)
                nc.vector.tensor_scalar_add(dcol[:], dcol[:], EPS)
                rcol = vec.tile([128, BPC], f32, tag="rcol", name=f"r{c}")
                st["r"] = rcol
                nc.vector.reciprocal(rcol[:], dcol[:])
            out.append(s_r)

            def s_S_b(bi):
                def fn():
                    if bi == 0:
                        st["Sps"] = psS.tile([128, H], f32, tag="S", name=f"S{c}")
                    sps = st["Sps"]
                    sl = slice(bi * 128, (bi + 1) * 128)
                    for hb in range(HB):
                        off = bi * H + hb * 128
                        nc.tensor.matmul(sps[:, sl], st["ktb"][:, off:off + 128],
                                         st["ktb"][:, off:off + 128],
                                         start=(hb == 0), stop=(hb == HB - 1))
                return fn
            for bi in range(BPC):
                out.append(s_S_b(bi))

            def s_A():
                a_t = chl.tile([128, H], bf16, tag="A", name=f"A{c}")
                st["A"] = a_t
                for bi in range(BPC):
                    sl = slice(bi * 128, (bi + 1) * 128)
                    nc.vector.scalar_tensor_tensor(
                        a_t[:, sl], st["Sps"][:, sl], st["r"][:, bi:bi + 1],
                        smask[:], mybir.AluOpType.mult, mybir.AluOpType.mult)
            out.append(s_A)

            def s_At():
                atp = pst.tile([128, PK // 2], bf16, tag="ktp")
                for bi in range(BPC):
                    sl = slice(bi * 128, (bi + 1) * 128)
                    nc.tensor.transpose(atp[:, sl], st["A"][:, sl], ident_b[:])
                at_t = chs.tile([128, H], bf16, tag="At")
                nc.scalar.copy(at_t[:], atp[:, :H])
                st["At"] = at_t
            out.append(s_At)

            late = []

            def s_B2():
                sq = psq.tile([128, H], f32, tag="sq")
                for bi in range(BPC):
                    sl = slice(bi * 128, (bi + 1) * 128)
                    nc.tensor.matmul(sq[:, sl], st["At"][:, sl], st["A"][:, sl],
                                     start=True, stop=True)
                b2 = chl.tile([128, H], bf16, tag="B2", name=f"B2{c}")
                nc.scalar.copy(b2[:], sq[:])
                st["B2"] = b2
            late.append(s_B2)

            def s_B2t():
                btp = pst.tile([128, PK // 2], bf16, tag="ktp")
                for bi in range(BPC):
                    sl = slice(bi * 128, (bi + 1) * 128)
                    nc.tensor.transpose(btp[:, sl], st["B2"][:, sl], ident_b[:])
                b2t = chs.tile([128, H], bf16, tag="B2t")
                nc.scalar.copy(b2t[:], btp[:, :H])
                st["B2t"] = b2t
            late.append(s_B2t)

            def s_B4():
                sq = psq.tile([128, H], f32, tag="sq")
                for bi in range(BPC):
                    sl = slice(bi * 128, (bi + 1) * 128)
                    nc.tensor.matmul(sq[:, sl], st["B2t"][:, sl], st["B2"][:, sl],
                                     start=True, stop=True)
                b4 = chl.tile([128, H], bf16, tag="B4", name=f"B4{c}")
                nc.scalar.copy(b4[:], sq[:])
                st["B4"] = b4
            late.append(s_B4)

            return out, late

        def serial_stages(c):
            """Backward-sweep stages for chunk c (consumes G[c])."""
            st = G[c]
            out = []
            hold = {}

            def s_p():
                pv = psv.tile([128, 128], f32, tag="pv")
                hold["pv"] = pv
                pb = vec.tile([128, HB * BPC], bf16, tag="pb")
                nc.vector.tensor_sub(pb[:], qcol[:], mcol[:])
                hold["pb"] = pb
            out.append(s_p)

            def s_rhs_b(bi):
                def fn():
                    pv = hold["pv"]
                    for hb in range(HB):
                        off = bi * H + hb * 128
                        nc.tensor.matmul(pv[:, bi:bi + 1],
                                         st["ktb"][:, off:off + 128],
                                         hold["pb"][:, bi * HB + hb:bi * HB + hb + 1],
                                         start=(bi == 0 and hb == 0),
                                         stop=(bi == BPC - 1 and hb == HB - 1))
                return fn
            for bi in range(BPC):
                out.append(s_rhs_b(bi))

            def s_y0():
                y0 = vec.tile([128, BPC], bf16, tag="y")
                nc.vector.tensor_copy(y0[:], hold["pv"][:, 0:BPC])
                hold["y"] = y0
            out.append(s_y0)

            def apply_factor(lev, mat_key, sub):
                cbase = 4 * lev

                def fn():
                    pv = hold["pv"]
                    for bi in range(BPC):
                        sl = slice(bi * 128, (bi + 1) * 128)
                        nc.tensor.matmul(pv[:, cbase + bi:cbase + bi + 1],
                                         st[mat_key][:, sl],
                                         hold["y"][:, bi:bi + 1],
                                         start=(bi == 0), stop=(bi == BPC - 1))
                    ynew = vec.tile([128, BPC], bf16, tag="y")
                    if sub:
                        nc.vector.tensor_sub(ynew[:], hold["y"][:],
                                             pv[:, cbase:cbase + BPC])
                    else:
                        nc.vector.tensor_add(ynew[:], hold["y"][:],
                                             pv[:, cbase:cbase + BPC])
                    hold["y"] = ynew
                return fn
            out.append(apply_factor(1, "A", True))
            out.append(apply_factor(2, "B2", False))
            out.append(apply_factor(3, "B4", False))
            if NLEV >= 3:
                out.append(apply_factor(4, "B8", False))

            def s_ry():
                # ryy: interleaved (ry, y) column pairs -> one matmul updates m+ctx
                ryy = vec.tile([128, 2 * BPC], bf16, tag="ryy")
                nc.vector.tensor_tensor(ryy[:, 0:2 * BPC:2], hold["y"][:], st["r"][:],
                                        mybir.AluOpType.mult)
                nc.vector.tensor_copy(ryy[:, 1:2 * BPC:2], hold["y"][:])
                hold["ryy"] = ryy
            out.append(s_ry)

            def s_mctx_b(bi):
                def fn():
                    pv = hold["pv"]
                    for hb in range(HB):
                        off = bi * H + hb * 128
                        col = 32 + 2 * (bi * HB + hb)
                        nc.tensor.matmul(pv[:, col:col + 2],
                                         st["kb"][:, off:off + 128],
                                         hold["ryy"][:, 2 * bi:2 * bi + 2],
                                         start=(bi == 0 and hb == 0),
                                         stop=(bi == BPC - 1 and hb == HB - 1))
                return fn
            for bi in range(BPC):
                out.append(s_mctx_b(bi))

            def s_macc():
                pv = hold["pv"]
                nc.vector.tensor_add(mcol[:], pv[:, 32:32 + 2 * HB * BPC:2], mcol[:])
                nc.vector.tensor_add(ctxcol[:], pv[:, 33:33 + 2 * HB * BPC:2], ctxcol[:])
            out.append(s_macc)

            return out

        # ---- emission: early prep (load..At) runs AHEAD_E chunks ahead,
        # late prep (B2..B4) runs AHEAD_L ahead, chain drains lead each round.
        AHEAD_E, AHEAD_L = 4, 2
        earlys, lates = {}, {}
        for cc in range(NCHUNK - 1, -1, -1):
            earlys[cc], lates[cc] = prep_stages(cc)
        for cc in range(NCHUNK - 1, NCHUNK - 1 - AHEAD_E, -1):
            for fn in earlys[cc]:
                fn()
            if cc >= NCHUNK - AHEAD_L:
                for fn in lates[cc]:
                    fn()
        for c in range(NCHUNK - 1, -1, -1):
            if c - AHEAD_L >= 0:
                for fn in lates[c - AHEAD_L]:
                    fn()
            sc = serial_stages(c)
            pc = list(earlys[c - AHEAD_E]) if c - AHEAD_E >= 0 else []
            if c == 11:
                pc = pc + wbuild_stages()
            n = max(len(sc), len(pc))
            for i in range(n):
                if i < len(sc):
                    sc[i]()
                lo = (i * len(pc)) // n
                hi = ((i + 1) * len(pc)) // n
                for j in range(lo, hi):
                    pc[j]()
            if c + 1 in G:
                del G[c + 1]

        # ---- finale: out_row = ctx W^T + b per batch ----
        ctxr = consts.tile([128, HB * BPC], f32r, tag="ctxr")
        nc.vector.tensor_copy(ctxr[:], ctxcol[:])
        for bi in range(BPC):
            ops_ = psS.tile([128, H], f32, tag="S")
            for ib in range(HB):
                nc.tensor.matmul(ops_[:1, :], ctxr[:, bi * HB + ib:bi * HB + ib + 1],
                                 wt[ib][:], start=(ib == 0), stop=(ib == HB - 1))
            out_row = small.tile([1, H], f32, tag="outrow")
            nc.vector.tensor_add(out_row[:], ops_[:1, :], bias_row[:])
            nc.sync.dma_start(out_d[bi, :][None, :], out_row[:])

    if legalize:
        _legalize_waits(nc)
    return nc


def _legalize_waits(nc, max_waits=1):
    """This toolchain's walrus encodes at most one semaphore wait per
    instruction. Hoist extra waits onto standalone EventSemaphore
    instructions on the same engine queue, immediately before the owner."""
    import json as _json
    m = _json.loads(bytes(nc.to_json_bytes()))
    n_fix = 0
    for fn in m["functions"]:
        for blk in fn["blocks"]:
            out = []
            for ins in blk.get("instructions", []):
                si = ins.get("sync_info") or {}
                waits = si.get("on_wait") or []
                if len(waits) > max_waits and ins.get("opcode") != "EventSemaphore":
                    extra, keep = waits[:-max_waits], waits[-max_waits:]
                    for i, w in enumerate(extra):
                        out.append({
                            "name": f"{ins['name']}-w{i}",
                            "engine": ins["engine"],
                            "opcode": "EventSemaphore",
                            "ins": [], "outs": [],
                            "sync_info": {"on_wait": [w], "on_update": []},
                        })
                    si["on_wait"] = keep
                    ins["sync_info"] = si
                    n_fix += 1
                out.append(ins)
            blk["instructions"] = out
    nc.m = mybir.module_from_json_bytes(_json.dumps(m).encode())
    return n_fix


def kernel(hidden: np.ndarray, W: np.ndarray, b: np.ndarray) -> np.ndarray:
    if "nc" not in _cached:
        _cached["nc"] = _build_program()
    nc = _cached["nc"]

    hidden = np.ascontiguousarray(hidden, dtype=np.float32)
    W = np.ascontiguousarray(W, dtype=np.float32)
    b = np.ascontiguousarray(b, dtype=np.float32)

    in_maps = []
    for ci in range(NCORES):
        in_maps.append({
            "hidden": hidden[ci * BPC:(ci + 1) * BPC],
            "W": W,
            "bvec": b,
        })
    res = run_bass_kernel_spmd(nc, in_maps, core_ids=list(range(NCORES)))
    _cached["last_results"] = res
    out = np.concatenate([res.results[ci]["out"] for ci in range(NCORES)], axis=0)
    return out.astype(np.float32)


if __name__ == "__main__":
    rng = np.random.default_rng(0)
    h = rng.standard_normal((B, L, H), dtype=np.float32)
    w = rng.standard_normal((H, H), dtype=np.float32) * (1.0 / np.sqrt(H))
    bb = np.zeros((H,), np.float32)
    o = kernel(h, w, bb)
    print(o.shape, o.dtype)
